# revision 4
# baseline (speedup 1.0000x reference)
"""CategorySpecificLinear Trainium2 kernel.

out[t] = x[t] @ weight[category_id[t]] + bias[category_id[t]]

Strategy: expert-parallel over the 8 categories (C == n_cores == 8).
On the host we route tokens by category (the "all-to-all" happens at
sharding time since we receive full inputs), transpose each category's
token block to [D, T_pad] (the PE needs the contraction dim on
partitions and fp32 has no DMA-transpose), and hand core c:
    xT   [D, T_pad]   tokens of category c, zero-padded to T_pad
    w    [D, O]       weight[c]
    bias [1, O]       bias[c]
Each core computes out = xT.T @ w + bias with fp32r matmuls (full fp32
precision at 1 col/cycle for N>=256), then the host scatters the rows
back to token order.

Per-core HBM traffic ~9 MB -> ~25 us at ~360 GB/s; PE work ~17 us
(overlapped), so the kernel is memory-bound as intended.
"""

import contextlib
import ctypes
import os
import sys
import types

import numpy as np

sys.path.insert(0, "/opt/trn_rl_repo")


def _ensure_ntff_hook():
    """Provide antenv.axon_hooks if the image lacks it.

    concourse.bass_utils imports antenv.axon_hooks.get_axon_ntff_profile_hook
    when trace=True under axon; some agent images don't ship that module, in
    which case the boot's NTFF hook registration silently degrades and the
    import in bass_utils crashes. Recreate the slim ctypes hook here
    (mirrors trn_agent_boot.trn_boot._ntff_profile_via_ctypes).
    """
    try:
        import antenv.axon_hooks  # noqa: F401

        return
    except ImportError:
        pass

    so_path = "/opt/axon/libaxon_pjrt.so"
    hook = None
    if os.path.exists(so_path):
        lib = ctypes.CDLL(so_path)
        if hasattr(lib, "axon_start_nrt_profile"):
            lib.axon_start_nrt_profile.argtypes = [
                ctypes.POINTER(ctypes.c_int64),
                ctypes.c_size_t,
            ]
            lib.axon_start_nrt_profile.restype = ctypes.c_int64
            lib.axon_stop_nrt_profile.argtypes = [ctypes.c_char_p]
            lib.axon_stop_nrt_profile.restype = ctypes.c_int64

            @contextlib.contextmanager
            def hook(output_dir, device_ids):
                import jax

                jax.devices()
                if device_ids:
                    ids = (ctypes.c_int64 * len(device_ids))(*device_ids)
                    rc = lib.axon_start_nrt_profile(ids, len(device_ids))
                else:
                    rc = lib.axon_start_nrt_profile(None, 0)
                if rc != 0:
                    raise RuntimeError(f"axon_start_nrt_profile rc={rc}")
                try:
                    yield
                finally:
                    n = lib.axon_stop_nrt_profile(str(output_dir).encode())
                    if n <= 0:
                        print(
                            f"ntff profile: rc={n} writing {output_dir}",
                            file=sys.stderr,
                        )

    mod = types.ModuleType("antenv.axon_hooks")
    _state = {"hook": hook}
    mod.set_axon_ntff_profile_hook = lambda h: _state.__setitem__("hook", h)
    mod.get_axon_ntff_profile_hook = lambda: _state["hook"]
    sys.modules["antenv.axon_hooks"] = mod
    try:
        import antenv

        antenv.axon_hooks = mod
    except ImportError:
        pass


_ensure_ntff_hook()

import concourse.bass as bass
import concourse.bacc as bacc_mod
import concourse.mybir as mybir
import concourse.tile as tile
from concourse.bass import ts
from concourse.bass_utils import run_bass_kernel_spmd

N_CORES = 8
P = 128
N_TILE = 512  # one fp32 PSUM bank; also >=256 keeps fp32r at full rate

_nc_cache = {}
LAST_RESULTS = None  # BassKernelResults of the most recent run (for test.py)


def _build_nc(T_pad: int, D: int, O: int):
    KO = D // P
    MO = T_pad // P
    NO = O // N_TILE
    mmdt = mybir.dt.float32r
    f32 = mybir.dt.float32

    nc = bacc_mod.Bacc()
    xT = nc.dram_tensor("xT", [D, T_pad], mmdt, kind="ExternalInput")
    w = nc.dram_tensor("w", [D, O], mmdt, kind="ExternalInput")
    bias = nc.dram_tensor("bias", [1, O], f32, kind="ExternalInput")
    out = nc.dram_tensor("out", [T_pad, O], f32, kind="ExternalOutput")

    xT_t = xT[:, :].rearrange("(ko p) t -> p ko t", p=P)
    w_t = w[:, :].rearrange("(ko p) o -> p ko o", p=P)
    out_t = out[:, :].rearrange("(mo p) o -> p mo o", p=P)

    with tile.TileContext(nc) as tc:
        with (
            tc.tile_pool(name="resident", bufs=1) as rpool,
            tc.tile_pool(name="psum", bufs=4, space="PSUM") as psum_pool,
            tc.tile_pool(name="obuf", bufs=4) as opool,
        ):
            bias_sb = rpool.tile([P, O], f32, tag="bias")
            nc.sync.dma_start(bias_sb[:], bias[:, :].to_broadcast((P, O)))

            # Interleave x / w loads k-major so the k-accumulation of the
            # first output tile can start as soon as k-slice 0 has landed.
            x_sb = []
            w_sb = {}
            for k in range(KO):
                xt = rpool.tile([P, T_pad], mmdt, tag=f"x{k}")
                nc.sync.dma_start(xt[:], xT_t[:, k, :])
                x_sb.append(xt)
                for n in range(NO):
                    wt = rpool.tile([P, N_TILE], mmdt, tag=f"w{k}_{n}")
                    nc.sync.dma_start(wt[:], w_t[:, k, ts(n, N_TILE)])
                    w_sb[(k, n)] = wt

            for m in range(MO):
                for n in range(NO):
                    ps = psum_pool.tile([P, N_TILE], f32)
                    for k in range(KO):
                        nc.tensor.matmul(
                            ps[:],
                            lhsT=x_sb[k][:, ts(m, P)],
                            rhs=w_sb[(k, n)][:],
                            start=(k == 0),
                            stop=(k == KO - 1),
                        )
                    ot = opool.tile([P, N_TILE], f32)
                    nc.vector.tensor_add(ot[:], ps[:], bias_sb[:, ts(n, N_TILE)])
                    nc.sync.dma_start(out_t[:, m, ts(n, N_TILE)], ot[:])
    nc.finalize()
    return nc


def kernel(x, category_id, weight, bias):
    global LAST_RESULTS
    x = np.asarray(x)
    category_id = np.asarray(category_id)
    weight = np.ascontiguousarray(np.asarray(weight), dtype=np.float32)
    bias = np.ascontiguousarray(np.asarray(bias), dtype=np.float32)

    orig_shape = x.shape
    D = orig_shape[-1]
    C, _, O = weight.shape
    assert C == N_CORES and D % P == 0 and O % N_TILE == 0

    T = int(np.prod(orig_shape[:-1]))
    x_flat = np.ascontiguousarray(x.reshape(T, D), dtype=np.float32)
    cid = category_id.reshape(T).astype(np.int64)

    idx_per_c = [np.flatnonzero(cid == c) for c in range(C)]
    counts = [len(ix) for ix in idx_per_c]
    T_pad = max(P, -(-max(counts) // P) * P)

    key = (T_pad, D, O)
    if key not in _nc_cache:
        _nc_cache[key] = _build_nc(T_pad, D, O)
    nc = _nc_cache[key]

    in_maps = []
    for c in range(C):
        xcT = np.zeros((D, T_pad), dtype=np.float32)
        xcT[:, : counts[c]] = x_flat[idx_per_c[c]].T
        in_maps.append(
            {
                "xT": xcT,
                "w": weight[c],
                "bias": bias[c : c + 1],
            }
        )

    res = run_bass_kernel_spmd(nc, in_maps, list(range(N_CORES)))
    LAST_RESULTS = res

    out_flat = np.empty((T, O), dtype=np.float32)
    for c in range(C):
        out_flat[idx_per_c[c]] = res.results[c]["out"][: counts[c]]
    return out_flat.reshape(*orig_shape[:-1], O)


# revision 7
# speedup vs baseline: 1.0911x; 1.0911x over previous
"""CategorySpecificLinear Trainium2 kernel.

out[t] = x[t] @ weight[category_id[t]] + bias[category_id[t]]

Strategy: expert-parallel over the 8 categories (C == n_cores == 8).
On the host we route tokens by category (the "all-to-all" happens at
sharding time since we receive full inputs), transpose each category's
token block to [D, T_pad] (the PE needs the contraction dim on
partitions and fp32 has no DMA-transpose), and hand core c:
    xT   [D, T_pad]   tokens of category c, zero-padded to T_pad
    w    [D, O]       weight[c]
    bias [1, O]       bias[c]
Each core computes out = xT.T @ w + bias with fp32r matmuls (full fp32
precision at 1 col/cycle for N>=256), then the host scatters the rows
back to token order.

Per-core HBM traffic ~9 MB -> ~25 us at ~360 GB/s; PE work ~17 us
(overlapped), so the kernel is memory-bound as intended.
"""

import contextlib
import ctypes
import os
import sys
import types

import numpy as np

sys.path.insert(0, "/opt/trn_rl_repo")


def _ensure_ntff_hook():
    """Provide antenv.axon_hooks if the image lacks it.

    concourse.bass_utils imports antenv.axon_hooks.get_axon_ntff_profile_hook
    when trace=True under axon; some agent images don't ship that module, in
    which case the boot's NTFF hook registration silently degrades and the
    import in bass_utils crashes. Recreate the slim ctypes hook here
    (mirrors trn_agent_boot.trn_boot._ntff_profile_via_ctypes).
    """
    try:
        import antenv.axon_hooks  # noqa: F401

        return
    except ImportError:
        pass

    so_path = "/opt/axon/libaxon_pjrt.so"
    hook = None
    if os.path.exists(so_path):
        lib = ctypes.CDLL(so_path)
        if hasattr(lib, "axon_start_nrt_profile"):
            lib.axon_start_nrt_profile.argtypes = [
                ctypes.POINTER(ctypes.c_int64),
                ctypes.c_size_t,
            ]
            lib.axon_start_nrt_profile.restype = ctypes.c_int64
            lib.axon_stop_nrt_profile.argtypes = [ctypes.c_char_p]
            lib.axon_stop_nrt_profile.restype = ctypes.c_int64

            @contextlib.contextmanager
            def hook(output_dir, device_ids):
                import jax

                jax.devices()
                if device_ids:
                    ids = (ctypes.c_int64 * len(device_ids))(*device_ids)
                    rc = lib.axon_start_nrt_profile(ids, len(device_ids))
                else:
                    rc = lib.axon_start_nrt_profile(None, 0)
                if rc != 0:
                    raise RuntimeError(f"axon_start_nrt_profile rc={rc}")
                try:
                    yield
                finally:
                    n = lib.axon_stop_nrt_profile(str(output_dir).encode())
                    if n <= 0:
                        print(
                            f"ntff profile: rc={n} writing {output_dir}",
                            file=sys.stderr,
                        )

    mod = types.ModuleType("antenv.axon_hooks")
    _state = {"hook": hook}
    mod.set_axon_ntff_profile_hook = lambda h: _state.__setitem__("hook", h)
    mod.get_axon_ntff_profile_hook = lambda: _state["hook"]
    sys.modules["antenv.axon_hooks"] = mod
    try:
        import antenv

        antenv.axon_hooks = mod
    except ImportError:
        pass


_ensure_ntff_hook()

import concourse.bass as bass
import concourse.bacc as bacc_mod
import concourse.mybir as mybir
import concourse.tile as tile
from concourse.bass import ts
from concourse.bass_utils import run_bass_kernel_spmd

N_CORES = 8
P = 128
N_TILE = 512  # one fp32 PSUM bank; also >=256 keeps fp32r at full rate

_nc_cache = {}
LAST_RESULTS = None  # BassKernelResults of the most recent run (for test.py)


def _build_nc(T_pad: int, D: int, O: int):
    KO = D // P
    MO = T_pad // P
    NO = O // N_TILE
    mmdt = mybir.dt.float32r
    f32 = mybir.dt.float32

    nc = bacc_mod.Bacc()
    xT = nc.dram_tensor("xT", [D, T_pad], mmdt, kind="ExternalInput")
    w = nc.dram_tensor("w", [D, O], mmdt, kind="ExternalInput")
    bias = nc.dram_tensor("bias", [1, O], f32, kind="ExternalInput")
    out = nc.dram_tensor("out", [T_pad, O], f32, kind="ExternalOutput")

    xT_t = xT[:, :].rearrange("(ko p) t -> p ko t", p=P)
    w_t = w[:, :].rearrange("(ko p) o -> p ko o", p=P)
    out_t = out[:, :].rearrange("(mo p) o -> p mo o", p=P)

    with tile.TileContext(nc) as tc:
        with (
            tc.tile_pool(name="resident", bufs=1) as rpool,
            tc.tile_pool(name="psum", bufs=8, space="PSUM") as psum_pool,
            tc.tile_pool(name="obuf", bufs=6) as opool,
        ):
            bias_sb = rpool.tile([P, O], f32, tag="bias")
            nc.scalar.dma_start(bias_sb[:], bias[:, :].to_broadcast((P, O)))

            # Loads split over the two HWDGE engines (SP: x, ACT: w) so the
            # ~650 ns per-dma_start issue cost doesn't serialize on one
            # engine; ordered k-major so wave A below can start as soon as
            # the first k-slice pair has landed.
            x_sb = []
            w_sb = {}
            for k in range(KO):
                xt = rpool.tile([P, T_pad], mmdt, tag=f"x{k}")
                nc.sync.dma_start(xt[:], xT_t[:, k, :])
                x_sb.append(xt)
                wt = rpool.tile([P, N_TILE], mmdt, tag=f"w{k}_0")
                nc.scalar.dma_start(wt[:], w_t[:, k, ts(0, N_TILE)])
                w_sb[(k, 0)] = wt
            for k in range(KO):
                for n in range(1, NO):
                    wt = rpool.tile([P, N_TILE], mmdt, tag=f"w{k}_{n}")
                    nc.scalar.dma_start(wt[:], w_t[:, k, ts(n, N_TILE)])
                    w_sb[(k, n)] = wt

            # One wave per n-tile: all MO psum groups accumulate in lockstep
            # over k, so the k-th step only needs x(k)/w(k,n) — PE starts
            # after the first ~600 KB instead of after the full 6.5 MB.
            for n in range(NO):
                pss = [
                    psum_pool.tile([P, N_TILE], f32, tag="ps", name=f"ps{n}_{m}")
                    for m in range(MO)
                ]
                for k in range(KO):
                    for m in range(MO):
                        nc.tensor.matmul(
                            pss[m][:],
                            lhsT=x_sb[k][:, ts(m, P)],
                            rhs=w_sb[(k, n)][:],
                            start=(k == 0),
                            stop=(k == KO - 1),
                        )
                for m in range(MO):
                    ot = opool.tile([P, N_TILE], f32)
                    nc.vector.tensor_add(ot[:], pss[m][:], bias_sb[:, ts(n, N_TILE)])
                    nc.gpsimd.dma_start(out_t[:, m, ts(n, N_TILE)], ot[:])
    nc.finalize()
    return nc


def kernel(x, category_id, weight, bias):
    global LAST_RESULTS
    x = np.asarray(x)
    category_id = np.asarray(category_id)
    weight = np.ascontiguousarray(np.asarray(weight), dtype=np.float32)
    bias = np.ascontiguousarray(np.asarray(bias), dtype=np.float32)

    orig_shape = x.shape
    D = orig_shape[-1]
    C, _, O = weight.shape
    assert C == N_CORES and D % P == 0 and O % N_TILE == 0

    T = int(np.prod(orig_shape[:-1]))
    x_flat = np.ascontiguousarray(x.reshape(T, D), dtype=np.float32)
    cid = category_id.reshape(T).astype(np.int64)

    idx_per_c = [np.flatnonzero(cid == c) for c in range(C)]
    counts = [len(ix) for ix in idx_per_c]
    T_pad = max(P, -(-max(counts) // P) * P)

    key = (T_pad, D, O)
    if key not in _nc_cache:
        _nc_cache[key] = _build_nc(T_pad, D, O)
    nc = _nc_cache[key]

    in_maps = []
    for c in range(C):
        xcT = np.zeros((D, T_pad), dtype=np.float32)
        xcT[:, : counts[c]] = x_flat[idx_per_c[c]].T
        in_maps.append(
            {
                "xT": xcT,
                "w": weight[c],
                "bias": bias[c : c + 1],
            }
        )

    res = run_bass_kernel_spmd(nc, in_maps, list(range(N_CORES)))
    LAST_RESULTS = res

    out_flat = np.empty((T, O), dtype=np.float32)
    for c in range(C):
        out_flat[idx_per_c[c]] = res.results[c]["out"][: counts[c]]
    return out_flat.reshape(*orig_shape[:-1], O)
